# revision 2
# baseline (speedup 1.0000x reference)
"""Multi-head attention with "restricted softmax" on 8 TRN2 NeuronCores.

Reference computation (per head):
    score = Q @ K.T / sqrt(D)                       # [S, S]
    attn  = exp(score) / (1 + sum_k exp(score))     # restricted softmax
            (mathematically identical to the max-clamped reference form)
    out   = attn @ V                                # [S, D]

Full problem: B=2, H=16, S=2048, D=64  ->  32 heads, 4 heads per core.

The ScalarEngine's exp is the hard floor (1 elem/cycle/lane @ 1.2 GHz,
(N+352)/1.2 ns per instruction), so the kernel is built to keep it
saturated with the widest ACTIVATEs PSUM allows:
  - Scores computed TRANSPOSED (S^T[k, q]) in [128, 512] units; THREE units
    share one ACTIVATE (N=1536 -> 1.025 ns/elem vs 1.12 at N=1024), using
    6 PSUM banks double-buffered + 2 banks for the PV accumulator.
  - Scores matmuls contract K=64 directly (no zero-padding: measured
    back-to-back K=64 matmuls stream at the same 216 ns/512-col as K=128).
  - PV uses lhsT=[V | 1] so PSUM row 64 accumulates sum_k exp (softmax
    denominator) for free.
  - NO TensorEngine transposes anywhere: Q/K staging transposes AND the
    output [d,q]->[q,d] epilogue transpose all run on the DMA X-bar
    (fp16 bounce through DRAM), keeping the PE stream pure matmul.
  - exp table is pre-warmed at kernel start so the ~2.7us ACT_TABLE_LOAD
    overlaps the initial DMA ramp.
"""

import os

import numpy as np

import concourse.bass as bass  # noqa: F401  (bass must import before tile)
import concourse.mybir as mybir
import concourse.tile as tile
from concourse import bacc
from concourse.bass_utils import run_bass_kernel_spmd

B, H, S, D = 2, 16, 2048, 64
N_CORES = 8
HPC = (B * H) // N_CORES  # heads per core = 4

F32 = mybir.dt.float32
F16 = mybir.dt.float16
EXP = mybir.ActivationFunctionType.Exp

SCALE = 1.0 / 8.0   # 1/sqrt(D)
NK = S // 128       # 16 k-tiles of 128
QB = 512            # q-block width per pass
NQB = S // QB       # 4 q-blocks per head
NPASS = HPC * NQB   # 16 passes
UNITS = NPASS * NK  # 256 scores units of [128k, 512q]
GRP = 3             # units per ACTIVATE group


class _HeadInputs:
    """Per-head staged inputs: fp16 Q^T/K^T [128, S] (only rows 0..63 are
    valid data; the scores matmul contracts K=64 directly) and [V | 1].

    Q^T/K^T are produced by a DMA X-bar transpose of an fp16 bounce buffer
    in DRAM: zero TensorEngine cost. `chunks` splits the chain so head 0's
    first q-block/k-tiles are ready ASAP during the ramp."""

    def __init__(self, ctx, h):
        self.ctx = ctx
        self.h = h

    def start_dma(self, chunks=1):
        nc, pools, h = self.ctx["nc"], self.ctx, self.h
        head_pool = pools["head_pool"]
        dram_pool = pools["dram_pool"]
        qkt_pool = pools["qkt_pool"]

        q_nat = head_pool.tile([128, NK, D], F32, tag="q_nat", name=f"q_nat{h}")
        k_nat = head_pool.tile([128, NK, D], F32, tag="k_nat", name=f"k_nat{h}")
        v_nat = head_pool.tile([128, NK, D], F32, tag="v_nat", name=f"v_nat{h}")
        # fp16 staging; cols 64..127 are never written nor read (the bounce
        # carries them as garbage, the transposed rows 64..127 are unused).
        q16 = head_pool.tile([128, NK, 128], F16, tag="q16", name=f"q16_{h}")
        k16 = head_pool.tile([128, NK, 128], F16, tag="k16", name=f"k16_{h}")
        qdr = dram_pool.tile([S, 128], F16, tag="qdr", name=f"qdr{h}")
        kdr = dram_pool.tile([S, 128], F16, tag="kdr", name=f"kdr{h}")
        self.qT = qkt_pool.tile([128, S], F16, tag="qT", name=f"qT{h}")
        self.kT = qkt_pool.tile([128, S], F16, tag="kT", name=f"kT{h}")

        nb = NK // chunks
        for c in range(chunks):
            ns = slice(c * nb, (c + 1) * nb)
            rows = slice(c * nb * 128, (c + 1) * nb * 128)
            for nat, st16, dr, tT, dram_src in (
                (k_nat, k16, kdr, self.kT, pools["k_dram"]),
                (q_nat, q16, qdr, self.qT, pools["q_dram"]),
            ):
                nc.sync.dma_start(
                    nat[:, ns, :],
                    dram_src[h].rearrange("(n p) d -> p n d", p=128)[:, ns, :],
                )
                nc.vector.tensor_copy(st16[:, ns, :D], nat[:, ns, :])
                nc.sync.dma_start(
                    dr[rows].rearrange("(n p) c -> p n c", p=128), st16[:, ns, :]
                )
                nc.sync.dma_start_transpose(tT[:, rows], dr[rows])

        nc.sync.dma_start(
            v_nat[:], pools["v_dram"][h].rearrange("(n p) d -> p n d", p=128)
        )
        v1 = head_pool.tile([128, NK, D + 1], F16, tag="v1", name=f"v1_{h}")
        nc.vector.tensor_copy(
            v1[:, :, D:].rearrange("p n one -> p (n one)"), pools["ones"][:]
        )
        nc.vector.tensor_copy(v1[:, :, :D], v_nat[:])
        self.v1 = v1


def _attention(tc):
    nc = tc.nc
    q_dram = nc.dram_tensor("query", [HPC, S, D], F32, kind="ExternalInput").ap()
    k_dram = nc.dram_tensor("key", [HPC, S, D], F32, kind="ExternalInput").ap()
    v_dram = nc.dram_tensor("value", [HPC, S, D], F32, kind="ExternalInput").ap()
    o_dram = nc.dram_tensor("out", [HPC, S, D], F32, kind="ExternalOutput").ap()

    with (
        tc.tile_pool(name="const", bufs=1) as const_pool,
        tc.tile_pool(name="head_io", bufs=2) as head_pool,
        tc.tile_pool(name="qkt", bufs=2) as qkt_pool,
        tc.tile_pool(name="et", bufs=2) as et_pool,
        tc.tile_pool(name="epi", bufs=2) as epi_pool,
        tc.tile_pool(name="dram", bufs=2, space="DRAM") as dram_pool,
        tc.tile_pool(name="dram_epi", bufs=2, space="DRAM") as dram_epi_pool,
        tc.tile_pool(name="ps_g", bufs=2, space="PSUM") as ps_g_pool,
        tc.tile_pool(name="ps_o", bufs=2, space="PSUM") as ps_o_pool,
    ):
        ones = const_pool.tile([128, NK], F16)
        nc.vector.memset(ones[:], 1.0)
        # pre-warm the exp table set so ACT_TABLE_LOAD overlaps the DMA ramp
        warm = const_pool.tile([128, 1], F16)
        nc.vector.memset(warm[:], 0.0)
        nc.scalar.activation(warm[:], warm[:], EXP)

        ctx = {
            "nc": nc, "q_dram": q_dram, "k_dram": k_dram, "v_dram": v_dram,
            "head_pool": head_pool, "qkt_pool": qkt_pool,
            "dram_pool": dram_pool, "ones": ones,
        }

        heads = [_HeadInputs(ctx, h) for h in range(HPC)]
        heads[0].start_dma(chunks=2)

        def emit_scores(units):
            slot = ps_g_pool.tile([128, GRP, QB], F32, tag="s", name="s")
            for j, u in enumerate(units):
                p, k = divmod(u, NK)
                h, qb = divmod(p, NQB)
                hd = heads[h]
                nc.tensor.matmul(
                    slot[:, j, :],
                    hd.kT[:64, k * 128:(k + 1) * 128],
                    hd.qT[:64, qb * QB:(qb + 1) * QB],
                    start=True, stop=True,
                )
            return slot

        def emit_epilogue(h, qb, oT):
            """Normalize + un-transpose oT [65, 512] -> out [512, 64] with
            zero TensorEngine work: fp16 bounce through DRAM, X-bar
            transpose back (chunk-major 3D dest), per-q reciprocal scale."""
            oT16 = epi_pool.tile([80, QB], F16, tag="oT16", name="oT16")
            nc.vector.tensor_copy(oT16[:65, :], oT[:])
            odr = dram_epi_pool.tile([80, QB], F16, tag="odr", name="odr")
            nc.sync.dma_start(odr[:65, :], oT16[:65, :])
            tr = epi_pool.tile([128, 4, 80], F16, tag="tr", name="tr")
            nc.sync.dma_start_transpose(tr[:], odr[:])
            den = epi_pool.tile([128, 4], F32, tag="den", name="den")
            nc.vector.tensor_scalar_add(den[:], tr[:, :, D], 1.0)
            rec = epi_pool.tile([128, 4], F32, tag="rec", name="rec")
            nc.vector.reciprocal(rec[:], den[:])
            o_sb = epi_pool.tile([128, 4, D], F32, tag="o_sb", name="o_sb")
            for j in range(4):
                nc.vector.tensor_scalar_mul(o_sb[:, j, :], tr[:, j, :D], rec[:, j:j + 1])
            nc.sync.dma_start(
                o_dram[h].rearrange("(n p) d -> p n d", p=128)[:, qb * 4:qb * 4 + 4, :],
                o_sb[:],
            )

        groups = [
            list(range(gs, min(gs + GRP, UNITS))) for gs in range(0, UNITS, GRP)
        ]
        slot_cur = emit_scores(groups[0])
        oT = None
        for g, units in enumerate(groups):
            w = len(units)
            et = et_pool.tile([128, GRP, QB], F16, tag="et", name="et")
            nc.scalar.activation(
                et[:, :w, :], slot_cur[:, :w, :], EXP, scale=SCALE
            )
            if g + 1 < len(groups):
                slot_cur = emit_scores(groups[g + 1])
            for j, u in enumerate(units):
                p, k = divmod(u, NK)
                h, qb = divmod(p, NQB)
                if k == 0:
                    oT = ps_o_pool.tile([65, QB], F32, tag="oT", name="oT")
                    # prefetch the next head's staging 3 passes ahead
                    if qb == 1 and h + 1 < HPC:
                        heads[h + 1].start_dma()
                nc.tensor.matmul(
                    oT[:], heads[h].v1[:, k, :], et[:, j, :],
                    start=(k == 0), stop=(k == NK - 1),
                )
                if k == NK - 1:
                    emit_epilogue(h, qb, oT)


_NC_CACHE = None
_TRACE_READY = False


def _enable_tracing():
    """Register the NTFF profile hook that this image's antenv lacks, and
    keep profiling artifacts local instead of uploading to a bucket."""
    global _TRACE_READY
    if _TRACE_READY:
        return
    import sys
    import types

    import antenv
    import concourse.bass_utils as bu
    from trn_agent_boot.trn_boot import _ntff_profile_via_ctypes

    if "antenv.axon_hooks" not in sys.modules:
        mod = types.ModuleType("antenv.axon_hooks")
        mod._hook = None

        def set_axon_ntff_profile_hook(h):
            mod._hook = h

        def get_axon_ntff_profile_hook():
            return mod._hook

        mod.set_axon_ntff_profile_hook = set_axon_ntff_profile_hook
        mod.get_axon_ntff_profile_hook = get_axon_ntff_profile_hook
        sys.modules["antenv.axon_hooks"] = mod
        antenv.axon_hooks = mod

    hooks = sys.modules["antenv.axon_hooks"]
    if hooks.get_axon_ntff_profile_hook() is None:
        hooks.set_axon_ntff_profile_hook(
            _ntff_profile_via_ctypes("/opt/axon/libaxon_pjrt.so")
        )
    bu.upload_artifacts = lambda tmpdir: tmpdir
    _TRACE_READY = True


def _build():
    global _NC_CACHE
    if _NC_CACHE is None:
        nc = bacc.Bacc("TRN2", target_bir_lowering=False, debug=False)
        with tile.TileContext(nc) as tc:
            _attention(tc)
        nc.compile()
        _NC_CACHE = nc
    return _NC_CACHE


def _run(query, key, value, trace=False, tmpdir=None):
    if trace:
        _enable_tracing()
    q = np.ascontiguousarray(np.asarray(query, dtype=np.float32).reshape(B * H, S, D))
    k = np.ascontiguousarray(np.asarray(key, dtype=np.float32).reshape(B * H, S, D))
    v = np.ascontiguousarray(np.asarray(value, dtype=np.float32).reshape(B * H, S, D))
    in_maps = [
        {
            "query": q[c * HPC:(c + 1) * HPC],
            "key": k[c * HPC:(c + 1) * HPC],
            "value": v[c * HPC:(c + 1) * HPC],
        }
        for c in range(N_CORES)
    ]
    nc = _build()
    res = run_bass_kernel_spmd(
        nc, in_maps, core_ids=list(range(N_CORES)), trace=trace, tmpdir=tmpdir
    )
    out = np.stack([res.results[c]["out"] for c in range(N_CORES)])  # [8, HPC, S, D]
    return out.reshape(B, H, S, D), res


def kernel(query, key, value):
    out, _ = _run(query, key, value, trace=bool(int(os.environ.get("BASS_TRACE", "0"))))
    return out
